# revision 13
# baseline (speedup 1.0000x reference)
"""Trainium2 Bass kernel for nn_EnhancedGraphConv (gnn_message_passing).

Strategy (8 cores): shard the B*N=1280 graph rows (b,i) as 160 rows/core
(cores 0-3 -> batch 0, 4-7 -> batch 1).  The HOST converts the dense
adjacency into padded neighbor lists (max degree <= 64 slots/row) and
gathers the referenced edge_features / neighbor x rows into compact
feature-major tensors, so the device streams dense [feat, token] tiles
with no on-device gather or transpose.  All matmuls run in float32r
(1 col/cycle at >=256 moving cols).  Three phases keep the activation
table stable (sigmoid set for the MLP phase, exp set for softmax and
the tail), and elementwise epilogues are spread across Act/DVE/Pool.
"""
import numpy as np
from contextlib import ExitStack

import concourse.bass as bass
import concourse.bacc as bacc
import concourse.tile as tile
from concourse import mybir
from concourse.bass_utils import run_bass_kernel_spmd
from concourse.masks import make_identity

F32 = mybir.dt.float32
F32R = mybir.dt.float32r
AF = mybir.ActivationFunctionType
OP = mybir.AluOpType

B, N, C, O, E = 2, 640, 64, 64, 18
D = 64            # neighbor slots per row
RG = 32           # rows per group
NCORES = 8
RPC = (B * N) // NCORES   # 160 rows per core
NG = RPC // RG            # 5 groups
TG = D * RG               # 2048 tokens per group
T = NG * TG               # 10240 tokens per core
CHUNK = 512
NCH = TG // CHUNK         # 4 chunks per group
NCOL = TG // 128          # 16 psc4 columns per group

# ---- weight blob column layouts ------------------------------------
# f32 blob: small-matmul weights, biases, per-row data
_BLOB = {}
_bw = 0
for _name, _p, _w in [
    ("Wxi", 64, 64), ("Wa3", 32, 1), ("Ws", 64, 64),
    ("Wc1", 128, 64), ("Wc2", 64, 64),
    ("be1", 64, 1), ("be2", 64, 1), ("be3", 32, 1), ("bhg", 128, 1),
    ("bn", 64, 1), ("ba2", 32, 1), ("bg2", 64, 1), ("bs", 64, 1),
    ("bc1", 64, 1), ("bc2", 64, 1),
    ("xr0", 128, 64), ("xr1", 32, 64),
    ("am0", 128, 64), ("am1", 32, 64),
]:
    _BLOB[_name] = (_p, _bw, _w)
    _bw += _w
BW = _bw
# f32r blob: weights consumed by full-rate fp32r matmuls
_RBLOB = {}
_rw = 0
for _name, _p, _w in [
    ("We1", 18, 64), ("We2", 64, 64), ("We3", 64, 32),
    ("Wpe", 32, 128), ("Wjj", 64, 128), ("Wn", 64, 64),
    ("W22", 128, 128), ("ind32", 32, 512), ("ones", 1, 64),
]:
    _RBLOB[_name] = (_p, _rw, _w)
    _rw += _w
RBW = _rw


def _build_nc(debug=False):
    nc = bacc.Bacc("TRN2", target_bir_lowering=False)
    t = {}
    t["blob"] = nc.dram_tensor("blob", [128, BW], F32, kind="ExternalInput")
    t["wrb"] = nc.dram_tensor("wrb", [128, RBW], F32R, kind="ExternalInput")
    t["ef"] = nc.dram_tensor("ef", [E, T], F32R, kind="ExternalInput")
    t["xj"] = nc.dram_tensor("xj", [C, T], F32R, kind="ExternalInput")
    t["out"] = nc.dram_tensor("out", [RPC, O], F32, kind="ExternalOutput")

    with tile.TileContext(nc) as tc, ExitStack() as ctx:
        w = ctx.enter_context(tc.tile_pool(name="w", bufs=1))
        strm = ctx.enter_context(tc.tile_pool(name="strm", bufs=2))
        mlp = ctx.enter_context(tc.tile_pool(name="mlp", bufs=4))
        wfl = ctx.enter_context(tc.tile_pool(name="wfl", bufs=2))
        p3s = ctx.enter_context(tc.tile_pool(name="p3s", bufs=3))
        prt = ctx.enter_context(tc.tile_pool(name="prt", bufs=8))
        sm = ctx.enter_context(tc.tile_pool(name="sm", bufs=4))
        ps = ctx.enter_context(tc.tile_pool(name="ps", bufs=3, space="PSUM"))
        pst = ctx.enter_context(tc.tile_pool(name="pst", bufs=1, space="PSUM"))
        pwbp = ctx.enter_context(tc.tile_pool(name="pwbp", bufs=2, space="PSUM"))
        pss = ctx.enter_context(tc.tile_pool(name="pss", bufs=2, space="PSUM"))

        ident = w.tile([128, 128], F32)
        make_identity(nc, ident[:])

        blob = w.tile([128, BW], F32, name="blob")
        nc.sync.dma_start(out=blob[:], in_=t["blob"][:])
        wrb = w.tile([128, RBW], F32R, name="wrb")
        nc.sync.dma_start(out=wrb[:], in_=t["wrb"][:])

        def wap(name):
            p, c0, wd = _BLOB[name]
            return blob[:p, c0:c0 + wd]

        def wr(name):
            p, c0, wd = _RBLOB[name]
            return wrb[:p, c0:c0 + wd]

        # ---- stage 0: per-node precomputes for this core's 160 rows
        xrf = w.tile([C, RPC], F32, name="xrf")       # x rows feature-major
        p0 = pst.tile([C, CHUNK], F32, name="ptr")
        nc.tensor.transpose(p0[:, :128], wap("xr0"), ident[:])
        nc.tensor.transpose(p0[:, 128:160], wap("xr1"), ident[:32, :32])
        nc.vector.tensor_copy(out=xrf[:], in_=p0[:, :RPC])
        axi = w.tile([C, RPC], F32, name="axi")       # Wa1[:C]^T x_i
        pa = pst.tile([C, CHUNK], F32, name="ptr")
        nc.tensor.matmul(pa[:, :RPC], wap("Wxi"), xrf[:], start=True, stop=True)
        nc.vector.tensor_copy(out=axi[:], in_=pa[:, :RPC])
        axiT = w.tile([RG, NG * C], F32R, name="axiT")  # per-group transposed
        for g in range(NG):
            pt = pst.tile([RG, CHUNK], F32, name="ptr")
            nc.tensor.transpose(pt[:, :C], axi[:, g * RG:(g + 1) * RG],
                                ident[:C, :C])
            nc.vector.tensor_copy(out=axiT[:, g * C:(g + 1) * C], in_=pt[:, :C])
        selff = w.tile([C, RPC], F32, name="selff")
        pb = pst.tile([C, CHUNK], F32, name="ptr")
        nc.tensor.matmul(pb[:, :RPC], wap("Ws"), xrf[:], start=True, stop=True)
        nc.scalar.activation(selff[:], pb[:, :RPC], AF.Identity, bias=wap("bs"))

        mdwA = w.tile([C, T], F32, name="mdwA")       # gates * (Wn x_j + bn)
        smA = w.tile([RG, NG * D], F32, name="smA")   # masked scores
        wTA = w.tile([D, NG * RG], F32R, name="wTA")   # transposed softmax w

        # ================= P1: per-edge MLPs + raw scores ==============
        for g in range(NG):
            gc = slice(g * TG, (g + 1) * TG)
            ef_g = strm.tile([E, TG], F32R, name="ef_g")
            nc.sync.dma_start(out=ef_g[:], in_=t["ef"][:, gc])
            xj_g = strm.tile([C, TG], F32R, name="xj_g")
            nc.sync.dma_start(out=xj_g[:], in_=t["xj"][:, gc])
            psc4 = pss.tile([128, NCOL], F32, name="sp")
            axiT_g = axiT[:, g * C:(g + 1) * C]
            for q in range(NCH):
                cols = slice(q * CHUNK, (q + 1) * CHUNK)
                # edge MLP layer 1/2/3
                ps1 = ps.tile([128, CHUNK], F32, name="mm")
                nc.tensor.matmul(ps1[:64, :], wr("We1"),
                                 ef_g[:, cols],
                                 start=True, stop=True)
                pe1 = mlp.tile([64, CHUNK], F32R, name="pe1")
                nc.scalar.activation(pe1[:], ps1[:64, :], AF.Relu,
                                     bias=wap("be1"))
                ps2 = ps.tile([128, CHUNK], F32, name="mm")
                nc.tensor.matmul(ps2[:64, :], wr("We2"), pe1[:],
                                 start=True, stop=True)
                pe2 = mlp.tile([64, CHUNK], F32R, name="pe2")
                nc.vector.tensor_scalar(out=pe2[:], in0=ps2[:64, :],
                                        scalar1=wap("be2"), scalar2=0.0,
                                        op0=OP.add, op1=OP.max)
                ps3 = ps.tile([128, CHUNK], F32, name="mm")
                nc.tensor.matmul(ps3[:32, :], wr("We3"), pe2[:],
                                 start=True, stop=True)
                pe3 = mlp.tile([32, CHUNK], F32R, name="pe3")
                nc.scalar.activation(pe3[:], ps3[:32, :], AF.Relu,
                                     bias=wap("be3"))
                # fused attention-h1 / gate-h1: Wpe^T pe3 + Wjj^T xj + xi-part
                ps4 = ps.tile([128, CHUNK], F32, name="mm")
                nc.tensor.matmul(ps4[:], wr("Wpe"), pe3[:],
                                 start=True, stop=False)
                nc.tensor.matmul(ps4[:64, :], axiT_g,
                                 wr("ind32"),
                                 start=False, stop=False, skip_group_check=True)
                nc.tensor.matmul(ps4[:], wr("Wjj"), xj_g[:, cols],
                                 start=False, stop=True)
                hg = mlp.tile([128, CHUNK], F32R, name="hg")
                nc.scalar.activation(hg[:], ps4[:], AF.Relu, bias=wap("bhg"))
                # second layer: h2 (attention) + gates
                ps6 = ps.tile([128, CHUNK], F32, name="mm")
                nc.tensor.matmul(ps6[:], wr("W22"), hg[:],
                                 start=True, stop=True)
                h2 = mlp.tile([32, CHUNK], F32, name="h2")
                nc.vector.tensor_scalar(out=h2[:], in0=ps6[:32, :],
                                        scalar1=wap("ba2"), scalar2=0.0,
                                        op0=OP.add, op1=OP.max)
                gates = mlp.tile([64, CHUNK], F32, name="gates")
                nc.scalar.activation(gates[:], ps6[64:128, :], AF.Sigmoid,
                                     bias=wap("bg2"))
                # tn = Wn^T xj + bn fused into mdwA = gates * tn
                ps5 = ps.tile([128, CHUNK], F32, name="mm")
                nc.tensor.matmul(ps5[:64, :], wr("Wn"),
                                 xj_g[:, cols],
                                 start=True, stop=True)
                nc.vector.scalar_tensor_tensor(
                    out=mdwA[:, g * TG + q * CHUNK:g * TG + (q + 1) * CHUNK],
                    in0=ps5[:64, :], scalar=wap("bn"), in1=gates[:],
                    op0=OP.add, op1=OP.mult)
                # attention scores, 4 d-slots per matmul
                for k in range(4):
                    c = q * 4 + k
                    nc.tensor.matmul(psc4[:, c:c + 1],
                                     h2[:, k * 128:(k + 1) * 128],
                                     wap("Wa3"), start=True, stop=True)
            # scores -> row-major [RG, D] fused with adjacency mask bias
            amg = (wap("am0")[g * RG:(g + 1) * RG, :] if g < 4
                   else wap("am1"))
            for pb_ in range(4):
                outap = bass.AP(tensor=smA.tensor,
                                offset=smA[:, g * D + pb_:g * D + pb_ + 1].offset,
                                ap=[smA[:].ap[0], [4, NCOL]])
                amap = bass.AP(tensor=amg.tensor,
                               offset=amg[:, pb_:pb_ + 1].offset,
                               ap=[amg.ap[0], [4, NCOL]])
                nc.vector.scalar_tensor_tensor(
                    out=outap, in0=psc4[pb_ * RG:(pb_ + 1) * RG, :],
                    scalar=0.0, in1=amap, op0=OP.add, op1=OP.add)

        # ================= P2: masked softmax per row ==================
        for g in range(NG):
            sc = smA[:, g * D:(g + 1) * D]
            negmax = sm.tile([RG, 1], F32, name="negmax")
            nc.vector.tensor_reduce(out=negmax[:], in_=sc,
                                    axis=mybir.AxisListType.X, op=OP.max,
                                    negate=True)
            pexp = sm.tile([RG, D + 1], F32, name="pexp")
            nc.scalar.activation(pexp[:, :D], sc, AF.Exp, bias=negmax[:],
                                 accum_out=pexp[:, D:D + 1])
            invz = sm.tile([RG, 1], F32, name="invz")
            nc.vector.tensor_scalar_add(out=invz[:], in0=pexp[:, D:D + 1],
                                        scalar1=1e-30)
            nc.vector.reciprocal(out=invz[:], in_=invz[:])
            wrow = sm.tile([RG, D], F32, name="wrow")
            nc.vector.tensor_scalar_mul(out=wrow[:], in0=pexp[:, :D],
                                        scalar1=invz[:])
            pwt = pss.tile([D, RG], F32, name="sp")
            nc.tensor.transpose(pwt[:], wrow[:], ident[:RG, :RG])
            nc.scalar.activation(wTA[:, g * RG:(g + 1) * RG], pwt[:], AF.Copy)

        # ================= P3: weighted messages + output MLP ==========
        for g in range(NG):
            wT = wTA[:, g * RG:(g + 1) * RG]
            wflat = wfl.tile([1, TG], F32R, name="wflat")
            nc.sync.dma_start(out=wflat[:, :TG // 2], in_=wT[:D // 2, :])
            nc.sync.dma_start(out=wflat[:, TG // 2:], in_=wT[D // 2:, :])
            comb = sm.tile([128, RG], F32, name="comb")
            nc.scalar.activation(comb[:64, :], selff[:, g * RG:(g + 1) * RG],
                                 AF.Copy)
            parts = []
            for q in range(NCH):
                cols = slice(q * CHUNK, (q + 1) * CHUNK)
                pwb = pwbp.tile([64, CHUNK], F32, name="pwb")
                nc.tensor.matmul(pwb[:], wr("ones"), wflat[:, cols],
                                 start=True, stop=True)
                wbs = p3s.tile([64, CHUNK], F32, name="wbs")
                nc.scalar.activation(wbs[:], pwb[:], AF.Copy)
                mdwB = p3s.tile([64, CHUNK], F32, name="mdwB")
                eng = nc.gpsimd if q < 2 else nc.vector
                eng.tensor_tensor(
                    out=mdwB[:], in0=mdwA[:, g * TG + q * CHUNK:
                                          g * TG + (q + 1) * CHUNK],
                    in1=wbs[:], op=OP.mult)
                part = prt.tile([64, RG], F32, name="part")
                mv = mdwB[:].rearrange("p (d r) -> p r d", d=CHUNK // RG)
                nc.vector.tensor_reduce(out=part[:], in_=mv,
                                        axis=mybir.AxisListType.X, op=OP.add)
                parts.append(part)
            nc.gpsimd.tensor_tensor(out=parts[0][:], in0=parts[0][:],
                                    in1=parts[1][:], op=OP.add)
            nc.gpsimd.tensor_tensor(out=parts[2][:], in0=parts[2][:],
                                    in1=parts[3][:], op=OP.add)
            nc.gpsimd.tensor_tensor(out=comb[64:128, :], in0=parts[0][:],
                                    in1=parts[2][:], op=OP.add)
            pc1 = pss.tile([64, RG], F32, name="sp")
            nc.tensor.matmul(pc1[:], wap("Wc1"), comb[:], start=True, stop=True)
            c1 = sm.tile([64, RG], F32, name="c1")
            nc.scalar.activation(c1[:], pc1[:], AF.Relu, bias=wap("bc1"))
            pc2 = pss.tile([64, RG], F32, name="sp")
            nc.tensor.matmul(pc2[:], wap("Wc2"), c1[:], start=True, stop=True)
            ofm = sm.tile([64, RG], F32, name="ofm")
            nc.scalar.activation(ofm[:], pc2[:], AF.Identity, bias=wap("bc2"))
            por = pss.tile([RG, 64], F32, name="sp")
            nc.tensor.transpose(por[:], ofm[:], ident[:64, :64])
            orow = sm.tile([RG, 64], F32, name="orow")
            nc.vector.tensor_copy(out=orow[:], in_=por[:])
            nc.sync.dma_start(out=t["out"][g * RG:(g + 1) * RG, :], in_=orow[:])
    nc.compile()
    return nc


_NC = None


def _host_prep(x, adjacency, edge_features, weights):
    """Build per-core input maps: weight blob + host-gathered feature-major
    edge/neighbor tensors."""
    adj = adjacency > 0
    order = np.argsort(~adj, axis=-1, kind="stable")   # [B, N, N]
    deg = adj.sum(-1)                                  # [B, N]
    assert deg.max() <= D, f"degree {deg.max()} exceeds {D} slots"
    jidx = order[:, :, :D].astype(np.int32)            # [B, N, D]
    slot = np.arange(D)[None, None, :]
    valid = slot < deg[:, :, None]
    jidx = np.where(valid, jidx, 0)
    am = np.where(valid, 0.0, -1e30).astype(np.float32)  # [B, N, D]

    Wa1, Wg1 = weights["Wa1"], weights["Wg1"]
    W22 = np.zeros((128, 128), np.float32)
    W22[:64, :32] = weights["Wa2"]
    W22[64:, 64:] = weights["Wg2"]
    ind32 = np.zeros((RG, CHUNK), np.float32)
    ind32[np.arange(CHUNK) % RG, np.arange(CHUNK)] = 1.0

    blob0 = np.zeros((128, BW), np.float32)
    wrb0 = np.zeros((128, RBW), np.float32)

    def put(name, arr):
        p, c0, wd = _BLOB[name]
        blob0[:p, c0:c0 + wd] = np.asarray(arr, np.float32).reshape(p, wd)

    def putr(name, arr):
        p, c0, wd = _RBLOB[name]
        wrb0[:p, c0:c0 + wd] = np.asarray(arr, np.float32).reshape(p, wd)

    putr("We1", weights["We1"]); putr("We2", weights["We2"])
    putr("We3", weights["We3"])
    putr("Wpe", np.concatenate([Wa1[2 * C:], Wg1[C:]], 1))
    putr("Wjj", np.concatenate([Wa1[C:2 * C], Wg1[:C]], 1))
    putr("Wn", weights["Wn"]); putr("W22", W22); putr("ind32", ind32)
    putr("ones", np.ones((1, 64), np.float32))
    put("Wxi", Wa1[:C])
    put("Wa3", weights["Wa3"]); put("Ws", weights["Ws"])
    put("Wc1", weights["Wc1"]); put("Wc2", weights["Wc2"])
    put("be1", weights["be1"][:, None]); put("be2", weights["be2"][:, None])
    put("be3", weights["be3"][:, None])
    put("bhg", np.concatenate([weights["ba1"], weights["bg1"]])[:, None])
    put("bn", weights["bn"][:, None]); put("ba2", weights["ba2"][:, None])
    put("bg2", weights["bg2"][:, None]); put("bs", weights["bs"][:, None])
    put("bc1", weights["bc1"][:, None]); put("bc2", weights["bc2"][:, None])

    in_maps = []
    for core in range(NCORES):
        b = core // 4
        i0 = (core % 4) * RPC
        blob = blob0.copy()
        rows = np.arange(i0, i0 + RPC)
        xr = x[b, rows].astype(np.float32)            # [160, 64]
        blob[:128, _BLOB["xr0"][1]:_BLOB["xr0"][1] + 64] = xr[:128]
        blob[:32, _BLOB["xr1"][1]:_BLOB["xr1"][1] + 64] = xr[128:]
        amr = am[b, rows]                             # [160, 64]
        blob[:128, _BLOB["am0"][1]:_BLOB["am0"][1] + 64] = amr[:128]
        blob[:32, _BLOB["am1"][1]:_BLOB["am1"][1] + 64] = amr[128:]

        # token order: group-major, then d-major within group (t = d*RG+r)
        jv = jidx[b, rows]                            # [160, 64]
        jt = jv.reshape(NG, RG, D).transpose(0, 2, 1).reshape(-1)   # [T]
        rt = np.broadcast_to(rows.reshape(NG, 1, RG),
                             (NG, D, RG)).reshape(-1)               # [T]
        ef_fm = np.ascontiguousarray(
            edge_features[b, rt, jt].astype(np.float32).T)          # [18, T]
        xj_fm = np.ascontiguousarray(x[b, jt].astype(np.float32).T)  # [64, T]
        in_maps.append({"blob": blob, "wrb": wrb0, "ef": ef_fm, "xj": xj_fm})
    return in_maps


def kernel(**inputs):
    global _NC
    x = np.asarray(inputs["x"], np.float32)
    adjacency = np.asarray(inputs["adjacency"], np.float32)
    edge_features = np.asarray(inputs["edge_features"], np.float32)
    weights = {k: np.asarray(v, np.float32) for k, v in inputs.items()
               if k not in ("x", "adjacency", "edge_features")}
    in_maps = _host_prep(x, adjacency, edge_features, weights)
    if _NC is None:
        _NC = _build_nc()
    res = run_bass_kernel_spmd(_NC, in_maps, list(range(NCORES)))
    out = np.zeros((B, N, O), np.float32)
    for core in range(NCORES):
        b = core // 4
        i0 = (core % 4) * RPC
        out[b, i0:i0 + RPC] = res.results[core]["out"]
    return out
